# revision 19
# baseline (speedup 1.0000x reference)
"""BitLinear forward on 8 Trainium2 NeuronCores.

out = (x_q @ w_q) * (beta * gamma)
  a      = mean(weight);  w_q = sign(weight - a)
  gamma  = max|x| per row; x_q = clip(x/(gamma+eps), -(1-eps), 1-eps)
  beta   = max|weight|

Sharding: data-parallel over rows of x (N=32768 -> 4096 rows/core),
weight (1024x1024) replicated; per-core scalar stats are computed
redundantly so no collectives are needed.

Kernel math note: since QB == 1, (x_q @ w_q)*beta*gamma equals
(x @ w_q) * beta * gamma/(gamma+eps) up to the +-(1-eps) clip.  The clip
only affects the row-max element by <=1e-5 relative, and gamma/(gamma+eps)
deviates from 1 by <= eps/gamma ~ 4e-6 -- both far below the bf16 rounding
used for the matmul (~2e-3).  So the kernel never materializes x_q or even
gamma; it feeds bf16(x) to the tensor engine and multiplies the output by
the scalar beta.

Layout note: the contraction dimension may be distributed over SBUF
partitions in ANY fixed permutation as long as x^T and w_q use the same
one.  This kernel loads w as [128, 8, 1024] with partition p holding the
8 consecutive rows 8p..8p+7 (32 KiB contiguous per partition => large
DMA packets => the 4 MiB load runs near HBM rate instead of the ~150
GB/s small-packet rate).  Matmul chunk r then contracts the 128 features
{8p + r}; the bf16 cast writes x de-interleaved ([128, 8, 128], feature
f at [q, f%8, f//8]) so each transpose stationary is a contiguous slice.

Timeline (per core):
  t~2-12   weight halves on the two HWDGE queues at full rate; x tiles
           0-1 trickle on the SWDGE queues; PE transposes them.
           Remaining x is gated behind the weight DMA (a dummy gpsimd
           copy depending on the second half) so the weight load - which
           gates every matmul through mean->sign - is never starved.
  t~12-14  per-chunk row sums (ACT accum_out + DVE reduces, mostly
           hidden under the DMA), mean via ones[128,128] matmul,
           8 PE warm transposes to re-ramp the HAM-throttled clock.
  t~14-26  signs land every ~1.07us (chunk 0 split in halves for a
           faster first unlock); chunk-major matmuls over 3
           pre-transposed tiles consume them without PE bubbles.
  steady   PE runs [T8(t+3), MM16(t)] back to back; DVE casts+evacuates,
           ACT scale-copies output halves, stores alternate between the
           sync and scalar HWDGE queues.  Last two tiles split their
           stores across both queues to cut the drain tail.
"""

import sys

import numpy as np

if "/opt/trn_rl_repo" not in sys.path:
    sys.path.insert(0, "/opt/trn_rl_repo")

N_CORES = 8
N_FEAT = 1024
N_OUT = 1024
P = 128
KC = N_FEAT // P  # 8 contraction chunks of 128
EPS = 1e-5
NTILE_SINGLE = 8  # tiles 0..7 load individually (early, for PE warm)

_NC_CACHE = {}
_PATCHED = False


def _split_multi_waits(nc, max_waits=1):
    """The walrus build in this image rejects instructions carrying more
    than one sync-wait ("Too many sync wait commands").  Tile's semaphore
    assignment attaches one wait per producer proc, so hoist surplus waits
    onto NOP carrier instructions inserted immediately before the waiting
    instruction on the same engine (waits execute before the instruction
    body, so this preserves semantics exactly)."""
    import bass_rust

    for fn in nc.m.functions:
        for blk in fn.blocks:
            insts = blk.instructions  # live list
            i = 0
            while i < len(insts):
                ins = insts[i]
                si = getattr(ins, "sync_info", None)
                if si is None:
                    i += 1
                    continue
                waits = list(si.on_wait)
                if len(waits) <= max_waits:
                    i += 1
                    continue
                keep = waits[:max_waits]
                surplus = waits[max_waits:]
                si.on_wait = keep
                carriers = []
                cur_list = nc.cur_bb.bb.instructions
                for j in range(0, len(surplus), max_waits):
                    nop = nc.engines[ins.engine].nop(nofuse=True)
                    nop.ins.sync_info = bass_rust.SyncInfo(
                        on_wait=surplus[j : j + max_waits], on_update=[]
                    )
                    popped = cur_list.pop()
                    assert popped is nop.ins
                    carriers.append(nop.ins)
                for k, c in enumerate(carriers):
                    insts.insert(i + k, c)
                i += len(carriers) + 1


def _patch_ldw_opt():
    """No-op: walrus's ldw-opt pass crashes codegen on this toolchain
    (visitInstLdweights), so the default-disabled flag stays disabled."""
    import concourse.bass_utils as bu

    if getattr(bu, "_ldw_opt_patched", False):
        return
    bu._ldw_opt_patched = True
    orig = bu.run_command

    def patched(cmd, **kw):
        if isinstance(cmd, list):
            cmd = [
                "--enable-ldw-opt=false" if False else c
                for c in cmd
            ]
        return orig(cmd, **kw)

    bu.run_command = patched


def _patch_tile_drain():
    global _PATCHED
    if _PATCHED:
        return
    _PATCHED = True
    import concourse.tile as tile

    orig = tile.TileContext._drain_and_barrier

    def patched(self, tick_clock, wait_clock):
        orig(self, tick_clock, wait_clock)
        _split_multi_waits(self.nc)

    tile.TileContext._drain_and_barrier = patched


def _build_nc(rows_per_core: int):
    import concourse.bass as bass
    import concourse.mybir as mybir
    import concourse.tile as tile

    _patch_tile_drain()

    f32 = mybir.dt.float32
    bf16 = mybir.dt.bfloat16
    R = rows_per_core
    assert R % (4 * P) == 0
    T = R // P           # 32 tiles of 128 rows
    NW = 3               # chunk-major warm window (tiles 0-2)
    KH = KC // 2         # 4 low chunks (contiguous), 4 high chunks (rowblock)

    nc = bass.Bass("TRN2", target_bir_lowering=False, debug=False)
    x_h = nc.declare_dram_parameter("x", [R, N_FEAT], f32, isOutput=False)
    w_h = nc.declare_dram_parameter("weight", [N_FEAT, N_OUT], f32, isOutput=False)
    i_h = nc.declare_dram_parameter("ident", [P, P], bf16, isOutput=False)
    o_h = nc.declare_dram_parameter("out", [R, N_OUT], f32, isOutput=True)

    # The weight loads in TWO layouts so all three DMA queues run with
    # their best packet size and the 4 MiB load finishes in ~7us:
    #  - rows 0-511 (contraction chunks 0-3) in chunk layout
    #    [p, c, n] = w[c*128+p, n]: 4 KiB lines on the two HWDGE queues
    #    (1 MiB each), contiguous transpose stationaries.
    #  - rows 512-1023 (chunks 4-7) in rowblock layout: partition p holds
    #    rows 512+4p..512+4p+3 = ONE 16 KiB run => big packets on the
    #    SWDGE queue (~400 GB/s).  Chunk 4+r contracts rows {512+4p+r};
    #    the matching transpose stationary reads x features at stride 4.
    wlo_ap = w_h[:, :].rearrange("(c p) n -> p c n", p=P)
    whi_ap = w_h[:, :].rearrange("(h p r) n -> h p (r n)", h=2, r=4)
    # x tiles 0-3 load individually behind the rowblock weight half on
    # the fast SWDGE queue; tiles 4+ come in 512-row groups (16 KiB runs)
    xt_ap = x_h[:, :].rearrange("(t q) n -> t q n", q=P)
    ot_ap = o_h[:, :].rearrange("(t q) n -> t q n", q=P)
    xg_ap = x_h[:, :].rearrange("(g q r) n -> g q (r n)", q=P, r=4)
    og_ap = o_h[:, :].rearrange("(g q r) n -> g r q n", q=P, r=4)

    with tile.TileContext(nc) as tc:
        with (
            tc.tile_pool(name="wpool", bufs=1) as wpool,
            tc.tile_pool(name="x1pool", bufs=4) as x1pool,
            tc.tile_pool(name="xgpool", bufs=3) as xgpool,
            tc.tile_pool(name="bpool", bufs=4) as bpool,
            tc.tile_pool(name="tpool", bufs=7) as tpool,
            tc.tile_pool(name="opool", bufs=6) as opool,
            tc.tile_pool(name="pspool", bufs=NW, space="PSUM") as pspool,
            tc.tile_pool(name="ps1pool", bufs=2, space="PSUM") as ps1pool,
        ):
            # ---- persistent weight-side tiles ----
            wlo = wpool.tile([P, KH, N_OUT], f32, tag="wlo")
            whi = wpool.tile([P, KH * N_OUT], f32, tag="whi")
            wqlo = wpool.tile([P, KH, N_OUT], bf16, tag="wqlo")
            wqhi = wpool.tile([P, KH * N_OUT], bf16, tag="wqhi")
            wsum = wpool.tile([P, KC], f32, tag="wsum")
            bmax = wpool.tile([P, KC], f32, tag="bmax")
            bmax1 = wpool.tile([P, 1], f32, tag="bmax1")
            pack2 = wpool.tile([1, 2], f32, tag="pack2")
            ones1 = wpool.tile([1, P], f32, tag="ones1")
            ssum = wpool.tile([P, 1], f32, tag="ssum")
            ones128 = wpool.tile([P, P], f32, tag="ones128")
            stats = wpool.tile([P, 2], f32, tag="stats")
            ident = wpool.tile([P, P], bf16, tag="ident")

            neg_a = stats[:, 0:1]
            beta = stats[:, 1:2]

            def w32sl(c, lo=0, hi=N_OUT):
                """f32 weight slice for contraction chunk c."""
                if c < KH:
                    return wlo[:, c, lo:hi]
                r = c - KH
                return whi[:, r * N_OUT + lo : r * N_OUT + hi]

            def wqsl(c, lo=0, hi=N_OUT):
                if c < KH:
                    return wqlo[:, c, lo:hi]
                r = c - KH
                return wqhi[:, r * N_OUT + lo : r * N_OUT + hi]

            # ---- doorbells first on all three queues ----
            nc.sync.dma_start(out=ident, in_=i_h[:, :])
            nc.gpsimd.dma_start(out=whi, in_=whi_ap[1, :, :])
            nc.sync.dma_start(out=wlo[:, 0, :], in_=wlo_ap[:, 0, :])
            nc.sync.dma_start(out=wlo[:, 1, :], in_=wlo_ap[:, 1, :])
            nc.scalar.dma_start(out=wlo[:, 2, :], in_=wlo_ap[:, 2, :])
            nc.scalar.dma_start(out=wlo[:, 3, :], in_=wlo_ap[:, 3, :])
            nc.vector.memset(ones128, 1.0)
            nc.vector.memset(ones1, 1.0)

            cur_group = [None]

            def emit_x_load(t):
                if t < 4:
                    x32 = x1pool.tile([P, N_FEAT], f32, tag="x32")
                    nc.gpsimd.dma_start(out=x32, in_=xt_ap[t, :, :])
                    return x32[:, :]
                g, r = divmod(t, 4)
                if r == 0:
                    xg = xgpool.tile([P, 4 * N_FEAT], f32, tag="xg", name=f"xg{g}")
                    nc.gpsimd.dma_start(out=xg, in_=xg_ap[g, :, :])
                    cur_group[0] = xg
                return cur_group[0][:, (t % 4) * N_FEAT : (t % 4 + 1) * N_FEAT]

            def emit_cast(src):
                xb = bpool.tile([P, N_FEAT], bf16, tag="xb")
                nc.vector.tensor_copy(out=xb, in_=src)
                return xb

            def emit_T(xb):
                xTps = ps1pool.tile([P, KC, P], bf16, tag="xTps")
                for c in range(KH):
                    nc.tensor.transpose(
                        xTps[:, c, :], xb[:, c * P : (c + 1) * P], ident
                    )
                # chunks 4-7 contract features {512 + 4m + r}: stride-4 read
                xbhi = xb[:, 512:1024].rearrange("q (m r) -> q r m", r=4)
                for r in range(4):
                    nc.tensor.transpose(xTps[:, KH + r, :], xbhi[:, r, :], ident)
                return xTps

            def emit_evac(xTps):
                xT = tpool.tile([P, KC, P], bf16, tag="xT")
                nc.vector.tensor_copy(out=xT, in_=xTps)
                return xT

            def emit_warm(n):
                warm_ps = ps1pool.tile([P, P], bf16, tag="xTps")
                for _ in range(n):
                    nc.tensor.transpose(warm_ps, ident, ident)

            def emit_mm(ps, xT):
                for c in range(KC):
                    for h in range(2):
                        nc.tensor.matmul(
                            ps[:, h * 512 : (h + 1) * 512],
                            xT[:, c, :],
                            wqsl(c, h * 512, (h + 1) * 512),
                            start=(c == 0),
                            stop=(c == KC - 1),
                        )

            def emit_out(t, ps, tail=False):
                o = opool.tile([P, N_OUT], f32, tag="o")
                if t < 4:
                    dst = ot_ap[t, :, :]
                else:
                    g, r = divmod(t, 4)
                    dst = og_ap[g, r, :, :]
                for h in range(2):
                    nc.scalar.activation(
                        out=o[:, h * 512 : (h + 1) * 512],
                        in_=ps[:, h * 512 : (h + 1) * 512],
                        func=mybir.ActivationFunctionType.Copy,
                        bias=0.0, scale=beta,
                    )
                if tail:
                    # partition-split across the sync queue and the (idle)
                    # gpsimd SWDGE queue; never ring bells on the busy ACT
                    nc.sync.dma_start(out=dst[0:64, :], in_=o[0:64, :])
                    nc.gpsimd.dma_start(out=dst[64:128, :], in_=o[64:128, :])
                else:
                    nc.sync.dma_start(out=dst, in_=o)

            # ---- x singles behind the rowblock weight on the fast queue ----
            srcs = {t: emit_x_load(t) for t in range(4)}
            xb0 = emit_cast(srcs.pop(0))
            emit_warm(4)
            xTps0 = emit_T(xb0)
            xT_list = {0: emit_evac(xTps0)}
            xb1 = emit_cast(srcs.pop(1))
            emit_warm(4)
            xT_list[1] = emit_evac(emit_T(xb1))

            # ---- row sums in arrival order (lo chunks trickle in on the
            # HWDGE queues; the hi half lands all at once ~15.7us) ----
            for c in (0, 2):
                nc.vector.tensor_reduce(
                    wsum[:, c : c + 1], w32sl(c),
                    axis=mybir.AxisListType.X, op=mybir.AluOpType.add,
                )
            for c in (1, 3):
                nc.scalar.activation(
                    out=wqsl(c), in_=w32sl(c),
                    func=mybir.ActivationFunctionType.Copy,
                    bias=0.0, scale=1.0,
                    accum_out=wsum[:, c : c + 1],
                )
            for c in (4, 5, 6):
                nc.vector.tensor_reduce(
                    wsum[:, c : c + 1], w32sl(c),
                    axis=mybir.AxisListType.X, op=mybir.AluOpType.add,
                )
            nc.scalar.activation(
                out=wqsl(7), in_=w32sl(7),
                func=mybir.ActivationFunctionType.Copy,
                bias=0.0, scale=1.0,
                accum_out=wsum[:, 7:8],
            )
            # tile 2's transpose fills the PE hole before the mean matmul
            xb2 = emit_cast(srcs.pop(2))
            xTps2 = emit_T(xb2)
            nc.vector.tensor_reduce(
                ssum, wsum, axis=mybir.AxisListType.X, op=mybir.AluOpType.add
            )
            na_ps = ps1pool.tile([P, 1], f32, tag="xTps")
            nc.tensor.matmul(na_ps, ones128, ssum, start=True, stop=True)
            nc.vector.tensor_scalar_mul(
                neg_a, na_ps, -1.0 / float(N_FEAT * N_OUT)
            )

            # ---- signs; chunk 0 in halves for a faster first unlock ----
            nc.scalar.activation(
                out=wqsl(0, 0, 512), in_=w32sl(0, 0, 512),
                func=mybir.ActivationFunctionType.Sign, bias=neg_a, scale=1.0,
            )
            nc.scalar.activation(
                out=wqsl(0, 512, 1024), in_=w32sl(0, 512, 1024),
                func=mybir.ActivationFunctionType.Sign, bias=neg_a, scale=1.0,
            )
            for c in range(1, KC):
                nc.scalar.activation(
                    out=wqsl(c), in_=w32sl(c),
                    func=mybir.ActivationFunctionType.Sign, bias=neg_a, scale=1.0,
                )

            # ---- warm matmuls: chunk-major over tiles 0-2 so each sign
            # feeds ~1.28us of PE work; tile 3's transposes slot into the
            # sign-gated bubbles ----
            xT_list[2] = emit_evac(xTps2)
            ps_w = [
                pspool.tile([P, N_OUT], f32, tag="ps", name=f"ps_w{i}")
                for i in range(NW)
            ]

            def warm_mm(c0, c1):
                for c in range(c0, c1):
                    for h in range(2):
                        for ti in range(NW):
                            nc.tensor.matmul(
                                ps_w[ti][:, h * 512 : (h + 1) * 512],
                                xT_list[ti][:, c, :],
                                wqsl(c, h * 512, (h + 1) * 512),
                                start=(c == 0),
                                stop=(c == KC - 1),
                            )

            warm_mm(0, 3)
            xb3 = emit_cast(srcs.pop(3))
            xT_list[3] = emit_evac(emit_T(xb3))
            warm_mm(3, 6)

            # ---- beta = max|w| (needed only by the first output copy) ----
            for c in range(KC):
                nc.vector.tensor_reduce(
                    bmax[:, c : c + 1], w32sl(c),
                    axis=mybir.AxisListType.X, op=mybir.AluOpType.max,
                    apply_absolute_value=True,
                )
            nc.vector.tensor_reduce(
                bmax1, bmax, axis=mybir.AxisListType.X, op=mybir.AluOpType.max
            )
            nc.gpsimd.tensor_reduce(
                pack2[:, 1:2], bmax1, axis=mybir.AxisListType.C,
                op=mybir.AluOpType.max,
            )
            b_ps = ps1pool.tile([P, 1], f32, tag="xTps")
            nc.tensor.matmul(b_ps, ones1, pack2[:, 1:2], start=True, stop=True)
            nc.vector.tensor_copy(out=beta, in_=b_ps)

            # ---- last two chunks tile-major with outputs interleaved so
            # PSUM frees as early as possible ----
            for ti in range(NW):
                for c in (6, 7):
                    for h in range(2):
                        nc.tensor.matmul(
                            ps_w[ti][:, h * 512 : (h + 1) * 512],
                            xT_list[ti][:, c, :],
                            wqsl(c, h * 512, (h + 1) * 512),
                            start=False,
                            stop=(c == KC - 1),
                        )
                emit_out(ti, ps_w[ti])

            xT_list[4] = emit_evac(emit_T(emit_cast(emit_x_load(4))))
            xT_list[5] = emit_evac(emit_T(emit_cast(emit_x_load(5))))

            # ---- steady loop: PE stream is [T8(t+3), MM16(t)] ----
            for t in range(NW, T):
                if t + 3 < T:
                    if t + 3 not in srcs:
                        srcs[t + 3] = emit_x_load(t + 3)
                    xT_list[t + 3] = emit_evac(emit_T(emit_cast(srcs.pop(t + 3))))
                xT = xT_list.pop(t)
                ps = pspool.tile([P, N_OUT], f32, tag="ps")
                emit_mm(ps, xT)
                emit_out(t, ps, tail=(t >= T - 2))

    return nc


def _get_nc(rows_per_core: int):
    if rows_per_core not in _NC_CACHE:
        _NC_CACHE[rows_per_core] = _build_nc(rows_per_core)
    return _NC_CACHE[rows_per_core]


def run(x, weight, trace=False, trace_cores=None):
    """Run on 8 cores; returns (out, BassKernelResults)."""
    from concourse.bass_utils import run_bass_kernel_spmd

    import ml_dtypes

    x = np.ascontiguousarray(np.asarray(x, dtype=np.float32))
    weight = np.ascontiguousarray(np.asarray(weight, dtype=np.float32))
    ident = np.eye(P, dtype=ml_dtypes.bfloat16)
    n = x.shape[0]
    assert n % N_CORES == 0
    rpc = n // N_CORES
    nc = _get_nc(rpc)
    in_maps = [
        {"x": x[i * rpc : (i + 1) * rpc], "weight": weight, "ident": ident}
        for i in range(N_CORES)
    ]
    kwargs = {}
    if trace:
        kwargs["trace"] = True
        if trace_cores is not None:
            kwargs["trace_cores"] = trace_cores
    res = run_bass_kernel_spmd(nc, in_maps, core_ids=list(range(N_CORES)), **kwargs)
    out = np.concatenate([r["out"] for r in res.results], axis=0)
    return out, res


def kernel(x, weight):
    out, _ = run(x, weight)
    return out


# revision 20
# speedup vs baseline: 1.0152x; 1.0152x over previous
"""BitLinear forward on 8 Trainium2 NeuronCores.

out = (x_q @ w_q) * (beta * gamma)
  a      = mean(weight);  w_q = sign(weight - a)
  gamma  = max|x| per row; x_q = clip(x/(gamma+eps), -(1-eps), 1-eps)
  beta   = max|weight|

Sharding: data-parallel over rows of x (N=32768 -> 4096 rows/core),
weight (1024x1024) replicated; per-core scalar stats are computed
redundantly so no collectives are needed.

Kernel math note: since QB == 1, (x_q @ w_q)*beta*gamma equals
(x @ w_q) * beta * gamma/(gamma+eps) up to the +-(1-eps) clip.  The clip
only affects the row-max element by <=1e-5 relative, and gamma/(gamma+eps)
deviates from 1 by <= eps/gamma ~ 4e-6 -- both far below the bf16 rounding
used for the matmul (~2e-3).  So the kernel never materializes x_q or even
gamma; it feeds bf16(x) to the tensor engine and multiplies the output by
the scalar beta.

Layout note: the contraction dimension may be distributed over SBUF
partitions in ANY fixed permutation as long as x^T and w_q use the same
one.  This kernel loads w as [128, 8, 1024] with partition p holding the
8 consecutive rows 8p..8p+7 (32 KiB contiguous per partition => large
DMA packets => the 4 MiB load runs near HBM rate instead of the ~150
GB/s small-packet rate).  Matmul chunk r then contracts the 128 features
{8p + r}; the bf16 cast writes x de-interleaved ([128, 8, 128], feature
f at [q, f%8, f//8]) so each transpose stationary is a contiguous slice.

Timeline (per core):
  t~2-12   weight halves on the two HWDGE queues at full rate; x tiles
           0-1 trickle on the SWDGE queues; PE transposes them.
           Remaining x is gated behind the weight DMA (a dummy gpsimd
           copy depending on the second half) so the weight load - which
           gates every matmul through mean->sign - is never starved.
  t~12-14  per-chunk row sums (ACT accum_out + DVE reduces, mostly
           hidden under the DMA), mean via ones[128,128] matmul,
           8 PE warm transposes to re-ramp the HAM-throttled clock.
  t~14-26  signs land every ~1.07us (chunk 0 split in halves for a
           faster first unlock); chunk-major matmuls over 3
           pre-transposed tiles consume them without PE bubbles.
  steady   PE runs [T8(t+3), MM16(t)] back to back; DVE casts+evacuates,
           ACT scale-copies output halves, stores alternate between the
           sync and scalar HWDGE queues.  Last two tiles split their
           stores across both queues to cut the drain tail.
"""

import sys

import numpy as np

if "/opt/trn_rl_repo" not in sys.path:
    sys.path.insert(0, "/opt/trn_rl_repo")

N_CORES = 8
N_FEAT = 1024
N_OUT = 1024
P = 128
KC = N_FEAT // P  # 8 contraction chunks of 128
EPS = 1e-5
NTILE_SINGLE = 8  # tiles 0..7 load individually (early, for PE warm)

_NC_CACHE = {}
_PATCHED = False


def _split_multi_waits(nc, max_waits=1):
    """The walrus build in this image rejects instructions carrying more
    than one sync-wait ("Too many sync wait commands").  Tile's semaphore
    assignment attaches one wait per producer proc, so hoist surplus waits
    onto NOP carrier instructions inserted immediately before the waiting
    instruction on the same engine (waits execute before the instruction
    body, so this preserves semantics exactly)."""
    import bass_rust

    for fn in nc.m.functions:
        for blk in fn.blocks:
            insts = blk.instructions  # live list
            i = 0
            while i < len(insts):
                ins = insts[i]
                si = getattr(ins, "sync_info", None)
                if si is None:
                    i += 1
                    continue
                waits = list(si.on_wait)
                if len(waits) <= max_waits:
                    i += 1
                    continue
                keep = waits[:max_waits]
                surplus = waits[max_waits:]
                si.on_wait = keep
                carriers = []
                cur_list = nc.cur_bb.bb.instructions
                for j in range(0, len(surplus), max_waits):
                    nop = nc.engines[ins.engine].nop(nofuse=True)
                    nop.ins.sync_info = bass_rust.SyncInfo(
                        on_wait=surplus[j : j + max_waits], on_update=[]
                    )
                    popped = cur_list.pop()
                    assert popped is nop.ins
                    carriers.append(nop.ins)
                for k, c in enumerate(carriers):
                    insts.insert(i + k, c)
                i += len(carriers) + 1


def _patch_ldw_opt():
    """No-op: walrus's ldw-opt pass crashes codegen on this toolchain
    (visitInstLdweights), so the default-disabled flag stays disabled."""
    import concourse.bass_utils as bu

    if getattr(bu, "_ldw_opt_patched", False):
        return
    bu._ldw_opt_patched = True
    orig = bu.run_command

    def patched(cmd, **kw):
        if isinstance(cmd, list):
            cmd = [
                "--enable-ldw-opt=false" if False else c
                for c in cmd
            ]
        return orig(cmd, **kw)

    bu.run_command = patched


def _patch_tile_drain():
    global _PATCHED
    if _PATCHED:
        return
    _PATCHED = True
    import concourse.tile as tile

    orig = tile.TileContext._drain_and_barrier

    def patched(self, tick_clock, wait_clock):
        orig(self, tick_clock, wait_clock)
        _split_multi_waits(self.nc)

    tile.TileContext._drain_and_barrier = patched


def _build_nc(rows_per_core: int):
    import concourse.bass as bass
    import concourse.mybir as mybir
    import concourse.tile as tile

    _patch_tile_drain()

    f32 = mybir.dt.float32
    bf16 = mybir.dt.bfloat16
    R = rows_per_core
    assert R % (4 * P) == 0
    T = R // P           # 32 tiles of 128 rows
    NW = 3               # chunk-major warm window (tiles 0-2)
    KH = KC // 2         # 4 low chunks (contiguous), 4 high chunks (rowblock)

    nc = bass.Bass("TRN2", target_bir_lowering=False, debug=False)
    x_h = nc.declare_dram_parameter("x", [R, N_FEAT], f32, isOutput=False)
    w_h = nc.declare_dram_parameter("weight", [N_FEAT, N_OUT], f32, isOutput=False)
    i_h = nc.declare_dram_parameter("ident", [P, P], bf16, isOutput=False)
    o_h = nc.declare_dram_parameter("out", [R, N_OUT], f32, isOutput=True)

    # The weight loads in TWO layouts so all three DMA queues run with
    # their best packet size and the 4 MiB load finishes in ~7us:
    #  - rows 0-511 (contraction chunks 0-3) in chunk layout
    #    [p, c, n] = w[c*128+p, n]: 4 KiB lines on the two HWDGE queues
    #    (1 MiB each), contiguous transpose stationaries.
    #  - rows 512-1023 (chunks 4-7) in rowblock layout: partition p holds
    #    rows 512+4p..512+4p+3 = ONE 16 KiB run => big packets on the
    #    SWDGE queue (~400 GB/s).  Chunk 4+r contracts rows {512+4p+r};
    #    the matching transpose stationary reads x features at stride 4.
    wlo_ap = w_h[:, :].rearrange("(c p) n -> p c n", p=P)
    whi_ap = w_h[:, :].rearrange("(h p r) n -> h p (r n)", h=2, r=4)
    # x tiles 0-3 load individually behind the rowblock weight half on
    # the fast SWDGE queue; tiles 4+ come in 512-row groups (16 KiB runs)
    xt_ap = x_h[:, :].rearrange("(t q) n -> t q n", q=P)
    ot_ap = o_h[:, :].rearrange("(t q) n -> t q n", q=P)
    xg_ap = x_h[:, :].rearrange("(g q r) n -> g q (r n)", q=P, r=4)
    og_ap = o_h[:, :].rearrange("(g q r) n -> g r q n", q=P, r=4)

    with tile.TileContext(nc) as tc:
        with (
            tc.tile_pool(name="wpool", bufs=1) as wpool,
            tc.tile_pool(name="x1pool", bufs=4) as x1pool,
            tc.tile_pool(name="xgpool", bufs=3) as xgpool,
            tc.tile_pool(name="bpool", bufs=4) as bpool,
            tc.tile_pool(name="tpool", bufs=7) as tpool,
            tc.tile_pool(name="opool", bufs=6) as opool,
            tc.tile_pool(name="pspool", bufs=NW, space="PSUM") as pspool,
            tc.tile_pool(name="ps1pool", bufs=2, space="PSUM") as ps1pool,
        ):
            # ---- persistent weight-side tiles ----
            wlo = wpool.tile([P, KH, N_OUT], f32, tag="wlo")
            whi = wpool.tile([P, KH * N_OUT], f32, tag="whi")
            wqlo = wpool.tile([P, KH, N_OUT], bf16, tag="wqlo")
            wqhi = wpool.tile([P, KH * N_OUT], bf16, tag="wqhi")
            wsum = wpool.tile([P, KC], f32, tag="wsum")
            bmax = wpool.tile([P, KC], f32, tag="bmax")
            bmax1 = wpool.tile([P, 1], f32, tag="bmax1")
            pack2 = wpool.tile([1, 2], f32, tag="pack2")
            ones1 = wpool.tile([1, P], f32, tag="ones1")
            ssum = wpool.tile([P, 1], f32, tag="ssum")
            ones128 = wpool.tile([P, P], f32, tag="ones128")
            stats = wpool.tile([P, 2], f32, tag="stats")
            ident = wpool.tile([P, P], bf16, tag="ident")

            neg_a = stats[:, 0:1]
            beta = stats[:, 1:2]

            def w32sl(c, lo=0, hi=N_OUT):
                """f32 weight slice for contraction chunk c."""
                if c < KH:
                    return wlo[:, c, lo:hi]
                r = c - KH
                return whi[:, r * N_OUT + lo : r * N_OUT + hi]

            def wqsl(c, lo=0, hi=N_OUT):
                if c < KH:
                    return wqlo[:, c, lo:hi]
                r = c - KH
                return wqhi[:, r * N_OUT + lo : r * N_OUT + hi]

            # ---- doorbells first on all three queues ----
            nc.sync.dma_start(out=ident, in_=i_h[:, :])
            HWH = KH * N_OUT // 2
            nc.gpsimd.dma_start(out=whi[:, 0:HWH], in_=whi_ap[1, :, 0:HWH])
            nc.gpsimd.dma_start(out=whi[:, HWH:], in_=whi_ap[1, :, HWH:])
            nc.sync.dma_start(out=wlo[:, 0, :], in_=wlo_ap[:, 0, :])
            nc.sync.dma_start(out=wlo[:, 1, :], in_=wlo_ap[:, 1, :])
            nc.scalar.dma_start(out=wlo[:, 2, :], in_=wlo_ap[:, 2, :])
            nc.scalar.dma_start(out=wlo[:, 3, :], in_=wlo_ap[:, 3, :])
            nc.vector.memset(ones128, 1.0)
            nc.vector.memset(ones1, 1.0)

            cur_group = [None]

            def emit_x_load(t):
                if t < 4:
                    x32 = x1pool.tile([P, N_FEAT], f32, tag="x32")
                    nc.gpsimd.dma_start(out=x32, in_=xt_ap[t, :, :])
                    return x32[:, :]
                g, r = divmod(t, 4)
                if r == 0:
                    xg = xgpool.tile([P, 4 * N_FEAT], f32, tag="xg", name=f"xg{g}")
                    nc.gpsimd.dma_start(out=xg, in_=xg_ap[g, :, :])
                    cur_group[0] = xg
                return cur_group[0][:, (t % 4) * N_FEAT : (t % 4 + 1) * N_FEAT]

            def emit_cast(src):
                xb = bpool.tile([P, N_FEAT], bf16, tag="xb")
                nc.vector.tensor_copy(out=xb, in_=src)
                return xb

            def emit_T(xb):
                xTps = ps1pool.tile([P, KC, P], bf16, tag="xTps")
                for c in range(KH):
                    nc.tensor.transpose(
                        xTps[:, c, :], xb[:, c * P : (c + 1) * P], ident
                    )
                # chunks 4-7 contract features {512 + 4m + r}: stride-4 read
                xbhi = xb[:, 512:1024].rearrange("q (m r) -> q r m", r=4)
                for r in range(4):
                    nc.tensor.transpose(xTps[:, KH + r, :], xbhi[:, r, :], ident)
                return xTps

            def emit_evac(xTps):
                xT = tpool.tile([P, KC, P], bf16, tag="xT")
                nc.vector.tensor_copy(out=xT, in_=xTps)
                return xT

            def emit_warm(n):
                warm_ps = ps1pool.tile([P, P], bf16, tag="xTps")
                for _ in range(n):
                    nc.tensor.transpose(warm_ps, ident, ident)

            def emit_mm(ps, xT):
                for c in range(KC):
                    for h in range(2):
                        nc.tensor.matmul(
                            ps[:, h * 512 : (h + 1) * 512],
                            xT[:, c, :],
                            wqsl(c, h * 512, (h + 1) * 512),
                            start=(c == 0),
                            stop=(c == KC - 1),
                        )

            def emit_out(t, ps, tail=False):
                o = opool.tile([P, N_OUT], f32, tag="o")
                if t < 4:
                    dst = ot_ap[t, :, :]
                else:
                    g, r = divmod(t, 4)
                    dst = og_ap[g, r, :, :]
                for h in range(2):
                    nc.scalar.activation(
                        out=o[:, h * 512 : (h + 1) * 512],
                        in_=ps[:, h * 512 : (h + 1) * 512],
                        func=mybir.ActivationFunctionType.Copy,
                        bias=0.0, scale=beta,
                    )
                if tail:
                    # partition-split across the sync queue and the (idle)
                    # gpsimd SWDGE queue; never ring bells on the busy ACT
                    nc.sync.dma_start(out=dst[0:64, :], in_=o[0:64, :])
                    nc.gpsimd.dma_start(out=dst[64:128, :], in_=o[64:128, :])
                else:
                    nc.sync.dma_start(out=dst, in_=o)

            # ---- x singles behind the rowblock weight on the fast queue ----
            srcs = {t: emit_x_load(t) for t in range(4)}
            xb0 = emit_cast(srcs.pop(0))
            emit_warm(4)
            xTps0 = emit_T(xb0)
            xT_list = {0: emit_evac(xTps0)}
            xb1 = emit_cast(srcs.pop(1))
            emit_warm(4)
            xT_list[1] = emit_evac(emit_T(xb1))

            # ---- row sums in arrival order (lo chunks trickle in on the
            # HWDGE queues; the hi half lands all at once ~15.7us) ----
            for c in (0, 2, 4, 6):
                nc.vector.tensor_reduce(
                    wsum[:, c : c + 1], w32sl(c),
                    axis=mybir.AxisListType.X, op=mybir.AluOpType.add,
                )
            for c in (1, 3, 5, 7):
                nc.scalar.activation(
                    out=wqsl(c), in_=w32sl(c),
                    func=mybir.ActivationFunctionType.Copy,
                    bias=0.0, scale=1.0,
                    accum_out=wsum[:, c : c + 1],
                )
            # tile 2's transpose fills the PE hole before the mean matmul
            xb2 = emit_cast(srcs.pop(2))
            xTps2 = emit_T(xb2)
            nc.vector.tensor_reduce(
                ssum, wsum, axis=mybir.AxisListType.X, op=mybir.AluOpType.add
            )
            na_ps = ps1pool.tile([P, 1], f32, tag="xTps")
            nc.tensor.matmul(na_ps, ones128, ssum, start=True, stop=True)
            nc.vector.tensor_scalar_mul(
                neg_a, na_ps, -1.0 / float(N_FEAT * N_OUT)
            )

            # ---- signs; chunk 0 in halves for a faster first unlock ----
            nc.scalar.activation(
                out=wqsl(0, 0, 512), in_=w32sl(0, 0, 512),
                func=mybir.ActivationFunctionType.Sign, bias=neg_a, scale=1.0,
            )
            nc.scalar.activation(
                out=wqsl(0, 512, 1024), in_=w32sl(0, 512, 1024),
                func=mybir.ActivationFunctionType.Sign, bias=neg_a, scale=1.0,
            )
            for c in range(1, KC):
                nc.scalar.activation(
                    out=wqsl(c), in_=w32sl(c),
                    func=mybir.ActivationFunctionType.Sign, bias=neg_a, scale=1.0,
                )

            # ---- warm matmuls: chunk-major over tiles 0-2 so each sign
            # feeds ~1.28us of PE work; tile 3's transposes slot into the
            # sign-gated bubbles ----
            xT_list[2] = emit_evac(xTps2)
            ps_w = [
                pspool.tile([P, N_OUT], f32, tag="ps", name=f"ps_w{i}")
                for i in range(NW)
            ]

            def warm_mm(c0, c1):
                for c in range(c0, c1):
                    for h in range(2):
                        for ti in range(NW):
                            nc.tensor.matmul(
                                ps_w[ti][:, h * 512 : (h + 1) * 512],
                                xT_list[ti][:, c, :],
                                wqsl(c, h * 512, (h + 1) * 512),
                                start=(c == 0),
                                stop=(c == KC - 1),
                            )

            warm_mm(0, 3)
            xb3 = emit_cast(srcs.pop(3))
            xT_list[3] = emit_evac(emit_T(xb3))
            warm_mm(3, 6)

            # ---- beta = max|w| (needed only by the first output copy) ----
            for c in range(KC):
                nc.vector.tensor_reduce(
                    bmax[:, c : c + 1], w32sl(c),
                    axis=mybir.AxisListType.X, op=mybir.AluOpType.max,
                    apply_absolute_value=True,
                )
            nc.vector.tensor_reduce(
                bmax1, bmax, axis=mybir.AxisListType.X, op=mybir.AluOpType.max
            )
            nc.gpsimd.tensor_reduce(
                pack2[:, 1:2], bmax1, axis=mybir.AxisListType.C,
                op=mybir.AluOpType.max,
            )
            b_ps = ps1pool.tile([P, 1], f32, tag="xTps")
            nc.tensor.matmul(b_ps, ones1, pack2[:, 1:2], start=True, stop=True)
            nc.vector.tensor_copy(out=beta, in_=b_ps)

            # ---- last two chunks tile-major with outputs interleaved so
            # PSUM frees as early as possible ----
            for ti in range(NW):
                for c in (6, 7):
                    for h in range(2):
                        nc.tensor.matmul(
                            ps_w[ti][:, h * 512 : (h + 1) * 512],
                            xT_list[ti][:, c, :],
                            wqsl(c, h * 512, (h + 1) * 512),
                            start=False,
                            stop=(c == KC - 1),
                        )
                emit_out(ti, ps_w[ti])

            xT_list[4] = emit_evac(emit_T(emit_cast(emit_x_load(4))))
            xT_list[5] = emit_evac(emit_T(emit_cast(emit_x_load(5))))

            # ---- steady loop: PE stream is [T8(t+3), MM16(t)] ----
            for t in range(NW, T):
                if t + 3 < T:
                    if t + 3 not in srcs:
                        srcs[t + 3] = emit_x_load(t + 3)
                    xT_list[t + 3] = emit_evac(emit_T(emit_cast(srcs.pop(t + 3))))
                xT = xT_list.pop(t)
                ps = pspool.tile([P, N_OUT], f32, tag="ps")
                if t == T - 1:
                    # final tile: finish the h0 half completely first so its
                    # copy+store overlap the h1 matmuls; stores split 4 ways
                    o = opool.tile([P, N_OUT], f32, tag="o")
                    g, r = divmod(t, 4)
                    dst = og_ap[g, r, :, :]
                    for h in range(2):
                        for c in range(KC):
                            nc.tensor.matmul(
                                ps[:, h * 512 : (h + 1) * 512],
                                xT[:, c, :],
                                wqsl(c, h * 512, (h + 1) * 512),
                                start=(c == 0),
                                stop=(c == KC - 1),
                            )
                        nc.scalar.activation(
                            out=o[:, h * 512 : (h + 1) * 512],
                            in_=ps[:, h * 512 : (h + 1) * 512],
                            func=mybir.ActivationFunctionType.Copy,
                            bias=0.0, scale=beta,
                        )
                        nc.sync.dma_start(
                            out=dst[0:64, h * 512 : (h + 1) * 512],
                            in_=o[0:64, h * 512 : (h + 1) * 512],
                        )
                        nc.gpsimd.dma_start(
                            out=dst[64:128, h * 512 : (h + 1) * 512],
                            in_=o[64:128, h * 512 : (h + 1) * 512],
                        )
                else:
                    emit_mm(ps, xT)
                    emit_out(t, ps, tail=(t >= T - 3))

    return nc


def _get_nc(rows_per_core: int):
    if rows_per_core not in _NC_CACHE:
        _NC_CACHE[rows_per_core] = _build_nc(rows_per_core)
    return _NC_CACHE[rows_per_core]


def run(x, weight, trace=False, trace_cores=None):
    """Run on 8 cores; returns (out, BassKernelResults)."""
    from concourse.bass_utils import run_bass_kernel_spmd

    import ml_dtypes

    x = np.ascontiguousarray(np.asarray(x, dtype=np.float32))
    weight = np.ascontiguousarray(np.asarray(weight, dtype=np.float32))
    ident = np.eye(P, dtype=ml_dtypes.bfloat16)
    n = x.shape[0]
    assert n % N_CORES == 0
    rpc = n // N_CORES
    nc = _get_nc(rpc)
    in_maps = [
        {"x": x[i * rpc : (i + 1) * rpc], "weight": weight, "ident": ident}
        for i in range(N_CORES)
    ]
    kwargs = {}
    if trace:
        kwargs["trace"] = True
        if trace_cores is not None:
            kwargs["trace_cores"] = trace_cores
    res = run_bass_kernel_spmd(nc, in_maps, core_ids=list(range(N_CORES)), **kwargs)
    out = np.concatenate([r["out"] for r in res.results], axis=0)
    return out, res


def kernel(x, weight):
    out, _ = run(x, weight)
    return out


# revision 22
# speedup vs baseline: 1.0173x; 1.0021x over previous
"""BitLinear forward on 8 Trainium2 NeuronCores.
out = (x_q @ w_q) * (beta * gamma)
  a      = mean(weight);  w_q = sign(weight - a)
  gamma  = max|x| per row; x_q = clip(x/(gamma+eps), -(1-eps), 1-eps)
  beta   = max|weight|

Sharding: data-parallel over rows of x (N=32768 -> 4096 rows/core),
weight (1024x1024) replicated; per-core scalar stats are computed
redundantly so no collectives are needed.

Kernel math note: since QB == 1, (x_q @ w_q)*beta*gamma equals
(x @ w_q) * beta * gamma/(gamma+eps) up to the +-(1-eps) clip.  The clip
only affects the row-max element by <=1e-5 relative, and gamma/(gamma+eps)
deviates from 1 by <= eps/gamma ~ 4e-6 -- both far below the bf16 rounding
used for the matmul (~2e-3).  So the kernel never materializes x_q or even
gamma; it feeds bf16(x) to the tensor engine and multiplies the output by
the scalar beta.

Layout note: the contraction dimension may be distributed over SBUF
partitions in ANY fixed permutation as long as x^T and w_q use the same
one.  This kernel loads w as [128, 8, 1024] with partition p holding the
8 consecutive rows 8p..8p+7 (32 KiB contiguous per partition => large
DMA packets => the 4 MiB load runs near HBM rate instead of the ~150
GB/s small-packet rate).  Matmul chunk r then contracts the 128 features
{8p + r}; the bf16 cast writes x de-interleaved ([128, 8, 128], feature
f at [q, f%8, f//8]) so each transpose stationary is a contiguous slice.

Timeline (per core):
  t~2-12   weight halves on the two HWDGE queues at full rate; x tiles
           0-1 trickle on the SWDGE queues; PE transposes them.
           Remaining x is gated behind the weight DMA (a dummy gpsimd
           copy depending on the second half) so the weight load - which
           gates every matmul through mean->sign - is never starved.
  t~12-14  per-chunk row sums (ACT accum_out + DVE reduces, mostly
           hidden under the DMA), mean via ones[128,128] matmul,
           8 PE warm transposes to re-ramp the HAM-throttled clock.
  t~14-26  signs land every ~1.07us (chunk 0 split in halves for a
           faster first unlock); chunk-major matmuls over 3
           pre-transposed tiles consume them without PE bubbles.
  steady   PE runs [T8(t+3), MM16(t)] back to back; DVE casts+evacuates,
           ACT scale-copies output halves, stores alternate between the
           sync and scalar HWDGE queues.  Last two tiles split their
           stores across both queues to cut the drain tail.
"""

import sys

import numpy as np

if "/opt/trn_rl_repo" not in sys.path:
    sys.path.insert(0, "/opt/trn_rl_repo")

N_CORES = 8
N_FEAT = 1024
N_OUT = 1024
P = 128
KC = N_FEAT // P  # 8 contraction chunks of 128
EPS = 1e-5
NTILE_SINGLE = 8  # tiles 0..7 load individually (early, for PE warm)

_NC_CACHE = {}
_PATCHED = False


def _split_multi_waits(nc, max_waits=1):
    """The walrus build in this image rejects instructions carrying more
    than one sync-wait ("Too many sync wait commands").  Tile's semaphore
    assignment attaches one wait per producer proc, so hoist surplus waits
    onto NOP carrier instructions inserted immediately before the waiting
    instruction on the same engine (waits execute before the instruction
    body, so this preserves semantics exactly)."""
    import bass_rust

    for fn in nc.m.functions:
        for blk in fn.blocks:
            insts = blk.instructions  # live list
            i = 0
            while i < len(insts):
                ins = insts[i]
                si = getattr(ins, "sync_info", None)
                if si is None:
                    i += 1
                    continue
                waits = list(si.on_wait)
                if len(waits) <= max_waits:
                    i += 1
                    continue
                keep = waits[:max_waits]
                surplus = waits[max_waits:]
                si.on_wait = keep
                carriers = []
                cur_list = nc.cur_bb.bb.instructions
                for j in range(0, len(surplus), max_waits):
                    nop = nc.engines[ins.engine].nop(nofuse=True)
                    nop.ins.sync_info = bass_rust.SyncInfo(
                        on_wait=surplus[j : j + max_waits], on_update=[]
                    )
                    popped = cur_list.pop()
                    assert popped is nop.ins
                    carriers.append(nop.ins)
                for k, c in enumerate(carriers):
                    insts.insert(i + k, c)
                i += len(carriers) + 1


def _patch_ldw_opt():
    """No-op: walrus's ldw-opt pass crashes codegen on this toolchain
    (visitInstLdweights), so the default-disabled flag stays disabled."""
    import concourse.bass_utils as bu

    if getattr(bu, "_ldw_opt_patched", False):
        return
    bu._ldw_opt_patched = True
    orig = bu.run_command

    def patched(cmd, **kw):
        if isinstance(cmd, list):
            cmd = [
                "--enable-ldw-opt=false" if False else c
                for c in cmd
            ]
        return orig(cmd, **kw)

    bu.run_command = patched


def _patch_tile_drain():
    global _PATCHED
    if _PATCHED:
        return
    _PATCHED = True
    import concourse.tile as tile

    orig = tile.TileContext._drain_and_barrier

    def patched(self, tick_clock, wait_clock):
        orig(self, tick_clock, wait_clock)
        _split_multi_waits(self.nc)

    tile.TileContext._drain_and_barrier = patched


def _build_nc(rows_per_core: int):
    import concourse.bass as bass
    import concourse.mybir as mybir
    import concourse.tile as tile

    _patch_tile_drain()

    f32 = mybir.dt.float32
    bf16 = mybir.dt.bfloat16
    R = rows_per_core
    assert R % (4 * P) == 0
    T = R // P           # 32 tiles of 128 rows
    NW = 3               # chunk-major warm window (tiles 0-2)
    KH = KC // 2         # 4 low chunks (contiguous), 4 high chunks (rowblock)

    nc = bass.Bass("TRN2", target_bir_lowering=False, debug=False)
    x_h = nc.declare_dram_parameter("x", [R, N_FEAT], f32, isOutput=False)
    w_h = nc.declare_dram_parameter("weight", [N_FEAT, N_OUT], f32, isOutput=False)
    i_h = nc.declare_dram_parameter("ident", [P, P], bf16, isOutput=False)
    o_h = nc.declare_dram_parameter("out", [R, N_OUT], f32, isOutput=True)

    # The weight loads in TWO layouts so all three DMA queues run with
    # their best packet size and the 4 MiB load finishes in ~7us:
    #  - rows 0-511 (contraction chunks 0-3) in chunk layout
    #    [p, c, n] = w[c*128+p, n]: 4 KiB lines on the two HWDGE queues
    #    (1 MiB each), contiguous transpose stationaries.
    #  - rows 512-1023 (chunks 4-7) in rowblock layout: partition p holds
    #    rows 512+4p..512+4p+3 = ONE 16 KiB run => big packets on the
    #    SWDGE queue (~400 GB/s).  Chunk 4+r contracts rows {512+4p+r};
    #    the matching transpose stationary reads x features at stride 4.
    wlo_ap = w_h[:, :].rearrange("(c p) n -> p c n", p=P)
    whi_ap = w_h[:, :].rearrange("(h p r) n -> h p (r n)", h=2, r=4)
    # x tiles 0-3 load individually behind the rowblock weight half on
    # the fast SWDGE queue; tiles 4+ come in 512-row groups (16 KiB runs)
    xt_ap = x_h[:, :].rearrange("(t q) n -> t q n", q=P)
    ot_ap = o_h[:, :].rearrange("(t q) n -> t q n", q=P)
    xg_ap = x_h[:, :].rearrange("(g q r) n -> g q (r n)", q=P, r=4)
    og_ap = o_h[:, :].rearrange("(g q r) n -> g r q n", q=P, r=4)

    with tile.TileContext(nc) as tc:
        with (
            tc.tile_pool(name="wpool", bufs=1) as wpool,
            tc.tile_pool(name="x1pool", bufs=8) as x1pool,
            tc.tile_pool(name="xgpool", bufs=3) as xgpool,
            tc.tile_pool(name="bpool", bufs=4) as bpool,
            tc.tile_pool(name="tpool", bufs=7) as tpool,
            tc.tile_pool(name="opool", bufs=6) as opool,
            tc.tile_pool(name="pspool", bufs=NW, space="PSUM") as pspool,
            tc.tile_pool(name="ps1pool", bufs=2, space="PSUM") as ps1pool,
        ):
            # ---- persistent weight-side tiles ----
            wlo = wpool.tile([P, KH, N_OUT], f32, tag="wlo")
            whi = wpool.tile([P, KH * N_OUT], f32, tag="whi")
            wqlo = wpool.tile([P, KH, N_OUT], bf16, tag="wqlo")
            wqhi = wpool.tile([P, KH * N_OUT], bf16, tag="wqhi")
            wsum = wpool.tile([P, KC], f32, tag="wsum")
            bmax = wpool.tile([P, KC], f32, tag="bmax")
            bmax1 = wpool.tile([P, 1], f32, tag="bmax1")
            pack2 = wpool.tile([1, 2], f32, tag="pack2")
            ones1 = wpool.tile([1, P], f32, tag="ones1")
            ssum = wpool.tile([P, 1], f32, tag="ssum")
            ones128 = wpool.tile([P, P], f32, tag="ones128")
            stats = wpool.tile([P, 2], f32, tag="stats")
            ident = wpool.tile([P, P], bf16, tag="ident")

            neg_a = stats[:, 0:1]
            beta = stats[:, 1:2]

            def w32sl(c, lo=0, hi=N_OUT):
                """f32 weight slice for contraction chunk c."""
                if c < KH:
                    return wlo[:, c, lo:hi]
                r = c - KH
                return whi[:, r * N_OUT + lo : r * N_OUT + hi]

            def wqsl(c, lo=0, hi=N_OUT):
                if c < KH:
                    return wqlo[:, c, lo:hi]
                r = c - KH
                return wqhi[:, r * N_OUT + lo : r * N_OUT + hi]

            # ---- doorbells first on all three queues ----
            nc.sync.dma_start(out=ident, in_=i_h[:, :])
            HWH = KH * N_OUT // 2
            nc.gpsimd.dma_start(out=whi[:, 0:HWH], in_=whi_ap[1, :, 0:HWH])
            nc.gpsimd.dma_start(out=whi[:, HWH:], in_=whi_ap[1, :, HWH:])
            nc.sync.dma_start(out=wlo[:, 0, :], in_=wlo_ap[:, 0, :])
            nc.sync.dma_start(out=wlo[:, 1, :], in_=wlo_ap[:, 1, :])
            nc.scalar.dma_start(out=wlo[:, 2, :], in_=wlo_ap[:, 2, :])
            nc.scalar.dma_start(out=wlo[:, 3, :], in_=wlo_ap[:, 3, :])
            nc.vector.memset(ones128, 1.0)
            nc.vector.memset(ones1, 1.0)

            cur_group = [None]

            def emit_x_load(t):
                if t < 4:
                    x32 = x1pool.tile([P, N_FEAT], f32, tag="x32")
                    nc.gpsimd.dma_start(out=x32, in_=xt_ap[t, :, :])
                    return x32[:, :]
                g, r = divmod(t, 4)
                if r == 0:
                    xg = xgpool.tile([P, 4 * N_FEAT], f32, tag="xg", name=f"xg{g}")
                    nc.gpsimd.dma_start(out=xg, in_=xg_ap[g, :, :])
                    cur_group[0] = xg
                return cur_group[0][:, (t % 4) * N_FEAT : (t % 4 + 1) * N_FEAT]

            def emit_cast(src):
                xb = bpool.tile([P, N_FEAT], bf16, tag="xb")
                nc.vector.tensor_copy(out=xb, in_=src)
                return xb

            def emit_T(xb):
                xTps = ps1pool.tile([P, KC, P], bf16, tag="xTps")
                for c in range(KH):
                    nc.tensor.transpose(
                        xTps[:, c, :], xb[:, c * P : (c + 1) * P], ident
                    )
                # chunks 4-7 contract features {512 + 4m + r}: stride-4 read
                xbhi = xb[:, 512:1024].rearrange("q (m r) -> q r m", r=4)
                for r in range(4):
                    nc.tensor.transpose(xTps[:, KH + r, :], xbhi[:, r, :], ident)
                return xTps

            def emit_evac(xTps):
                xT = tpool.tile([P, KC, P], bf16, tag="xT")
                nc.vector.tensor_copy(out=xT, in_=xTps)
                return xT

            def emit_warm(n):
                warm_ps = ps1pool.tile([P, P], bf16, tag="xTps")
                for _ in range(n):
                    nc.tensor.transpose(warm_ps, ident, ident)

            def emit_mm(ps, xT):
                for c in range(KC):
                    for h in range(2):
                        nc.tensor.matmul(
                            ps[:, h * 512 : (h + 1) * 512],
                            xT[:, c, :],
                            wqsl(c, h * 512, (h + 1) * 512),
                            start=(c == 0),
                            stop=(c == KC - 1),
                        )

            def emit_out(t, ps, tail=False):
                o = opool.tile([P, N_OUT], f32, tag="o")
                if t < 4:
                    dst = ot_ap[t, :, :]
                else:
                    g, r = divmod(t, 4)
                    dst = og_ap[g, r, :, :]
                for h in range(2):
                    nc.scalar.activation(
                        out=o[:, h * 512 : (h + 1) * 512],
                        in_=ps[:, h * 512 : (h + 1) * 512],
                        func=mybir.ActivationFunctionType.Copy,
                        bias=0.0, scale=beta,
                    )
                if tail:
                    # partition-split across the sync queue and the (idle)
                    # gpsimd SWDGE queue; never ring bells on the busy ACT
                    nc.sync.dma_start(out=dst[0:64, :], in_=o[0:64, :])
                    nc.gpsimd.dma_start(out=dst[64:128, :], in_=o[64:128, :])
                else:
                    nc.sync.dma_start(out=dst, in_=o)

            # ---- x singles behind the rowblock weight on the fast queue,
            # loaded in feature-halves: the c0-c3 transposes (and with them
            # the first matmul) unlock ~2us after the first half lands ----
            def emit_half_chain(t):
                xa = x1pool.tile([P, 512], f32, tag="x32", name=f"xa{t}")
                nc.gpsimd.dma_start(out=xa, in_=xt_ap[t, :, 0:512])
                xbt = bpool.tile([P, N_FEAT], bf16, tag="xb")
                nc.vector.tensor_copy(out=xbt[:, 0:512], in_=xa)
                xTps = ps1pool.tile([P, KC, P], bf16, tag="xTps")
                for c in range(KH):
                    nc.tensor.transpose(
                        xTps[:, c, :], xbt[:, c * P : (c + 1) * P], ident
                    )
                xb_ = x1pool.tile([P, 512], f32, tag="x32", name=f"xb{t}")
                nc.gpsimd.dma_start(out=xb_, in_=xt_ap[t, :, 512:1024])
                nc.vector.tensor_copy(out=xbt[:, 512:1024], in_=xb_)
                xbhi = xbt[:, 512:1024].rearrange("q (m r) -> q r m", r=4)
                for r in range(4):
                    nc.tensor.transpose(xTps[:, KH + r, :], xbhi[:, r, :], ident)
                return emit_evac(xTps)

            srcs = {}
            xT_list = {}
            xT_list[0] = emit_half_chain(0)
            emit_warm(4)
            xT_list[1] = emit_half_chain(1)

            # ---- row sums in arrival order (lo chunks trickle in on the
            # HWDGE queues; the hi half lands all at once ~15.7us) ----
            for c in (0, 2, 4, 6):
                nc.vector.tensor_reduce(
                    wsum[:, c : c + 1], w32sl(c),
                    axis=mybir.AxisListType.X, op=mybir.AluOpType.add,
                )
            for c in (1, 3, 5, 7):
                nc.scalar.activation(
                    out=wqsl(c), in_=w32sl(c),
                    func=mybir.ActivationFunctionType.Copy,
                    bias=0.0, scale=1.0,
                    accum_out=wsum[:, c : c + 1],
                )
            # tile 2's transposes fill the PE hole before the mean matmul
            xT_list[2] = emit_half_chain(2)
            nc.vector.tensor_reduce(
                ssum, wsum, axis=mybir.AxisListType.X, op=mybir.AluOpType.add
            )
            na_ps = ps1pool.tile([P, 1], f32, tag="xTps")
            nc.tensor.matmul(na_ps, ones128, ssum, start=True, stop=True)
            nc.vector.tensor_scalar_mul(
                neg_a, na_ps, -1.0 / float(N_FEAT * N_OUT)
            )

            # ---- signs; chunk 0 in halves for a faster first unlock ----
            nc.scalar.activation(
                out=wqsl(0, 0, 512), in_=w32sl(0, 0, 512),
                func=mybir.ActivationFunctionType.Sign, bias=neg_a, scale=1.0,
            )
            nc.scalar.activation(
                out=wqsl(0, 512, 1024), in_=w32sl(0, 512, 1024),
                func=mybir.ActivationFunctionType.Sign, bias=neg_a, scale=1.0,
            )
            for c in range(1, KC):
                nc.scalar.activation(
                    out=wqsl(c), in_=w32sl(c),
                    func=mybir.ActivationFunctionType.Sign, bias=neg_a, scale=1.0,
                )

            # ---- warm matmuls: chunk-major over tiles 0-2 so each sign
            # feeds ~1.28us of PE work; tile 3's transposes slot into the
            # sign-gated bubbles ----
            ps_w = [
                pspool.tile([P, N_OUT], f32, tag="ps", name=f"ps_w{i}")
                for i in range(NW)
            ]

            def warm_mm(c0, c1):
                for c in range(c0, c1):
                    for h in range(2):
                        for ti in range(NW):
                            nc.tensor.matmul(
                                ps_w[ti][:, h * 512 : (h + 1) * 512],
                                xT_list[ti][:, c, :],
                                wqsl(c, h * 512, (h + 1) * 512),
                                start=(c == 0),
                                stop=(c == KC - 1),
                            )

            warm_mm(0, 3)
            xT_list[3] = emit_half_chain(3)
            warm_mm(3, 6)



            # ---- last two chunks tile-major with outputs interleaved so
            # PSUM frees as early as possible ----
            for ti in range(NW):
                for c in (6, 7):
                    for h in range(2):
                        nc.tensor.matmul(
                            ps_w[ti][:, h * 512 : (h + 1) * 512],
                            xT_list[ti][:, c, :],
                            wqsl(c, h * 512, (h + 1) * 512),
                            start=False,
                            stop=(c == KC - 1),
                        )
                emit_out(ti, ps_w[ti])

            xT_list[4] = emit_evac(emit_T(emit_cast(emit_x_load(4))))
            xT_list[5] = emit_evac(emit_T(emit_cast(emit_x_load(5))))

            # ---- beta = max|w| (needed only by the first output copy) ----
            for c in range(KC):
                nc.vector.tensor_reduce(
                    bmax[:, c : c + 1], w32sl(c),
                    axis=mybir.AxisListType.X, op=mybir.AluOpType.max,
                    apply_absolute_value=True,
                )
            nc.vector.tensor_reduce(
                bmax1, bmax, axis=mybir.AxisListType.X, op=mybir.AluOpType.max
            )
            nc.gpsimd.tensor_reduce(
                pack2[:, 1:2], bmax1, axis=mybir.AxisListType.C,
                op=mybir.AluOpType.max,
            )
            b_ps = ps1pool.tile([P, 1], f32, tag="xTps")
            nc.tensor.matmul(b_ps, ones1, pack2[:, 1:2], start=True, stop=True)
            nc.vector.tensor_copy(out=beta, in_=b_ps)



            # ---- steady loop: PE stream is [T8(t+3), MM16(t)] ----
            for t in range(NW, T):
                if t + 3 < T:
                    if t + 3 not in srcs:
                        srcs[t + 3] = emit_x_load(t + 3)
                    xT_list[t + 3] = emit_evac(emit_T(emit_cast(srcs.pop(t + 3))))
                xT = xT_list.pop(t)
                ps = pspool.tile([P, N_OUT], f32, tag="ps")
                if t == T - 1:
                    # final tile: finish the h0 half completely first so its
                    # copy+store overlap the h1 matmuls; stores split 4 ways
                    o = opool.tile([P, N_OUT], f32, tag="o")
                    g, r = divmod(t, 4)
                    dst = og_ap[g, r, :, :]
                    for h in range(2):
                        for c in range(KC):
                            nc.tensor.matmul(
                                ps[:, h * 512 : (h + 1) * 512],
                                xT[:, c, :],
                                wqsl(c, h * 512, (h + 1) * 512),
                                start=(c == 0),
                                stop=(c == KC - 1),
                            )
                        nc.scalar.activation(
                            out=o[:, h * 512 : (h + 1) * 512],
                            in_=ps[:, h * 512 : (h + 1) * 512],
                            func=mybir.ActivationFunctionType.Copy,
                            bias=0.0, scale=beta,
                        )
                        nc.sync.dma_start(
                            out=dst[0:64, h * 512 : (h + 1) * 512],
                            in_=o[0:64, h * 512 : (h + 1) * 512],
                        )
                        nc.gpsimd.dma_start(
                            out=dst[64:128, h * 512 : (h + 1) * 512],
                            in_=o[64:128, h * 512 : (h + 1) * 512],
                        )
                else:
                    emit_mm(ps, xT)
                    emit_out(t, ps, tail=(t >= T - 3))

    return nc


def _get_nc(rows_per_core: int):
    if rows_per_core not in _NC_CACHE:
        _NC_CACHE[rows_per_core] = _build_nc(rows_per_core)
    return _NC_CACHE[rows_per_core]


def run(x, weight, trace=False, trace_cores=None):
    """Run on 8 cores; returns (out, BassKernelResults)."""
    from concourse.bass_utils import run_bass_kernel_spmd

    import ml_dtypes

    x = np.ascontiguousarray(np.asarray(x, dtype=np.float32))
    weight = np.ascontiguousarray(np.asarray(weight, dtype=np.float32))
    ident = np.eye(P, dtype=ml_dtypes.bfloat16)
    n = x.shape[0]
    assert n % N_CORES == 0
    rpc = n // N_CORES
    nc = _get_nc(rpc)
    in_maps = [
        {"x": x[i * rpc : (i + 1) * rpc], "weight": weight, "ident": ident}
        for i in range(N_CORES)
    ]
    kwargs = {}
    if trace:
        kwargs["trace"] = True
        if trace_cores is not None:
            kwargs["trace_cores"] = trace_cores
    res = run_bass_kernel_spmd(nc, in_maps, core_ids=list(range(N_CORES)), **kwargs)
    out = np.concatenate([r["out"] for r in res.results], axis=0)
    return out, res


def kernel(x, weight):
    out, _ = run(x, weight)
    return out


# revision 24
# speedup vs baseline: 1.0268x; 1.0093x over previous
"""BitLinear forward on 8 Trainium2 NeuronCores.
out = (x_q @ w_q) * (beta * gamma)
  a      = mean(weight);  w_q = sign(weight - a)
  gamma  = max|x| per row; x_q = clip(x/(gamma+eps), -(1-eps), 1-eps)
  beta   = max|weight|

Sharding: data-parallel over rows of x (N=32768 -> 4096 rows/core),
weight (1024x1024) replicated; per-core scalar stats are computed
redundantly so no collectives are needed.

Kernel math note: since QB == 1, (x_q @ w_q)*beta*gamma equals
(x @ w_q) * beta * gamma/(gamma+eps) up to the +-(1-eps) clip.  The clip
only affects the row-max element by <=1e-5 relative, and gamma/(gamma+eps)
deviates from 1 by <= eps/gamma ~ 4e-6 -- both far below the bf16 rounding
used for the matmul (~2e-3).  So the kernel never materializes x_q or even
gamma; it feeds bf16(x) to the tensor engine and multiplies the output by
the scalar beta.

Layout note: the contraction dimension may be distributed over SBUF
partitions in ANY fixed permutation as long as x^T and w_q use the same
one.  This kernel loads w as [128, 8, 1024] with partition p holding the
8 consecutive rows 8p..8p+7 (32 KiB contiguous per partition => large
DMA packets => the 4 MiB load runs near HBM rate instead of the ~150
GB/s small-packet rate).  Matmul chunk r then contracts the 128 features
{8p + r}; the bf16 cast writes x de-interleaved ([128, 8, 128], feature
f at [q, f%8, f//8]) so each transpose stationary is a contiguous slice.

Timeline (per core):
  t~2-12   weight halves on the two HWDGE queues at full rate; x tiles
           0-1 trickle on the SWDGE queues; PE transposes them.
           Remaining x is gated behind the weight DMA (a dummy gpsimd
           copy depending on the second half) so the weight load - which
           gates every matmul through mean->sign - is never starved.
  t~12-14  per-chunk row sums (ACT accum_out + DVE reduces, mostly
           hidden under the DMA), mean via ones[128,128] matmul,
           8 PE warm transposes to re-ramp the HAM-throttled clock.
  t~14-26  signs land every ~1.07us (chunk 0 split in halves for a
           faster first unlock); chunk-major matmuls over 3
           pre-transposed tiles consume them without PE bubbles.
  steady   PE runs [T8(t+3), MM16(t)] back to back; DVE casts+evacuates,
           ACT scale-copies output halves, stores alternate between the
           sync and scalar HWDGE queues.  Last two tiles split their
           stores across both queues to cut the drain tail.
"""

import sys

import numpy as np

if "/opt/trn_rl_repo" not in sys.path:
    sys.path.insert(0, "/opt/trn_rl_repo")

N_CORES = 8
N_FEAT = 1024
N_OUT = 1024
P = 128
KC = N_FEAT // P  # 8 contraction chunks of 128
EPS = 1e-5
NTILE_SINGLE = 8  # tiles 0..7 load individually (early, for PE warm)

_NC_CACHE = {}
_PATCHED = False


def _split_multi_waits(nc, max_waits=1):
    """The walrus build in this image rejects instructions carrying more
    than one sync-wait ("Too many sync wait commands").  Tile's semaphore
    assignment attaches one wait per producer proc, so hoist surplus waits
    onto NOP carrier instructions inserted immediately before the waiting
    instruction on the same engine (waits execute before the instruction
    body, so this preserves semantics exactly)."""
    import bass_rust

    for fn in nc.m.functions:
        for blk in fn.blocks:
            insts = blk.instructions  # live list
            i = 0
            while i < len(insts):
                ins = insts[i]
                si = getattr(ins, "sync_info", None)
                if si is None:
                    i += 1
                    continue
                waits = list(si.on_wait)
                if len(waits) <= max_waits:
                    i += 1
                    continue
                keep = waits[:max_waits]
                surplus = waits[max_waits:]
                si.on_wait = keep
                carriers = []
                cur_list = nc.cur_bb.bb.instructions
                for j in range(0, len(surplus), max_waits):
                    nop = nc.engines[ins.engine].nop(nofuse=True)
                    nop.ins.sync_info = bass_rust.SyncInfo(
                        on_wait=surplus[j : j + max_waits], on_update=[]
                    )
                    popped = cur_list.pop()
                    assert popped is nop.ins
                    carriers.append(nop.ins)
                for k, c in enumerate(carriers):
                    insts.insert(i + k, c)
                i += len(carriers) + 1


def _patch_ldw_opt():
    """No-op: walrus's ldw-opt pass crashes codegen on this toolchain
    (visitInstLdweights), so the default-disabled flag stays disabled."""
    import concourse.bass_utils as bu

    if getattr(bu, "_ldw_opt_patched", False):
        return
    bu._ldw_opt_patched = True
    orig = bu.run_command

    def patched(cmd, **kw):
        if isinstance(cmd, list):
            cmd = [
                "--enable-ldw-opt=false" if False else c
                for c in cmd
            ]
        return orig(cmd, **kw)

    bu.run_command = patched


def _patch_tile_drain():
    global _PATCHED
    if _PATCHED:
        return
    _PATCHED = True
    import concourse.tile as tile

    orig = tile.TileContext._drain_and_barrier

    def patched(self, tick_clock, wait_clock):
        orig(self, tick_clock, wait_clock)
        _split_multi_waits(self.nc)

    tile.TileContext._drain_and_barrier = patched


def _build_nc(rows_per_core: int):
    import concourse.bass as bass
    import concourse.mybir as mybir
    import concourse.tile as tile

    _patch_tile_drain()

    f32 = mybir.dt.float32
    bf16 = mybir.dt.bfloat16
    R = rows_per_core
    assert R % (4 * P) == 0
    T = R // P           # 32 tiles of 128 rows
    NW = 3               # chunk-major warm window (tiles 0-2)
    KH = KC // 2         # 4 low chunks (contiguous), 4 high chunks (rowblock)

    nc = bass.Bass("TRN2", target_bir_lowering=False, debug=False)
    x_h = nc.declare_dram_parameter("x", [R, N_FEAT], f32, isOutput=False)
    w_h = nc.declare_dram_parameter("weight", [N_FEAT, N_OUT], f32, isOutput=False)
    i_h = nc.declare_dram_parameter("ident", [P, P], bf16, isOutput=False)
    o_h = nc.declare_dram_parameter("out", [R, N_OUT], f32, isOutput=True)

    # The weight loads in TWO layouts so all three DMA queues run with
    # their best packet size and the 4 MiB load finishes in ~7us:
    #  - rows 0-511 (contraction chunks 0-3) in chunk layout
    #    [p, c, n] = w[c*128+p, n]: 4 KiB lines on the two HWDGE queues
    #    (1 MiB each), contiguous transpose stationaries.
    #  - rows 512-1023 (chunks 4-7) in rowblock layout: partition p holds
    #    rows 512+4p..512+4p+3 = ONE 16 KiB run => big packets on the
    #    SWDGE queue (~400 GB/s).  Chunk 4+r contracts rows {512+4p+r};
    #    the matching transpose stationary reads x features at stride 4.
    wlo_ap = w_h[:, :].rearrange("(c p) n -> p c n", p=P)
    whi_ap = w_h[:, :].rearrange("(h p r) n -> h p (r n)", h=2, r=4)
    # x tiles 0-3 load individually behind the rowblock weight half on
    # the fast SWDGE queue; tiles 4+ come in 512-row groups (16 KiB runs)
    xt_ap = x_h[:, :].rearrange("(t q) n -> t q n", q=P)
    ot_ap = o_h[:, :].rearrange("(t q) n -> t q n", q=P)
    xg_ap = x_h[:, :].rearrange("(g q r) n -> g q (r n)", q=P, r=4)
    og_ap = o_h[:, :].rearrange("(g q r) n -> g r q n", q=P, r=4)

    with tile.TileContext(nc) as tc:
        with (
            tc.tile_pool(name="wpool", bufs=1) as wpool,
            tc.tile_pool(name="x1pool", bufs=8) as x1pool,
            tc.tile_pool(name="xgpool", bufs=3) as xgpool,
            tc.tile_pool(name="bpool", bufs=4) as bpool,
            tc.tile_pool(name="tpool", bufs=7) as tpool,
            tc.tile_pool(name="opool", bufs=6) as opool,
            tc.tile_pool(name="pspool", bufs=NW, space="PSUM") as pspool,
            tc.tile_pool(name="ps1pool", bufs=2, space="PSUM") as ps1pool,
        ):
            # ---- persistent weight-side tiles ----
            wlo = wpool.tile([P, KH, N_OUT], f32, tag="wlo")
            whi = wpool.tile([P, KH * N_OUT], f32, tag="whi")
            wqlo = wpool.tile([P, KH, N_OUT], bf16, tag="wqlo")
            wqhi = wpool.tile([P, KH * N_OUT], bf16, tag="wqhi")
            wsum = wpool.tile([P, KC], f32, tag="wsum")
            bmax = wpool.tile([P, KC], f32, tag="bmax")
            bmax1 = wpool.tile([P, 1], f32, tag="bmax1")
            pack2 = wpool.tile([1, 2], f32, tag="pack2")
            ones1 = wpool.tile([1, P], f32, tag="ones1")
            ssum = wpool.tile([P, 1], f32, tag="ssum")
            ones128 = wpool.tile([P, P], f32, tag="ones128")
            stats = wpool.tile([P, 2], f32, tag="stats")
            ident = wpool.tile([P, P], bf16, tag="ident")

            neg_a = stats[:, 0:1]
            beta = stats[:, 1:2]

            def w32sl(c, lo=0, hi=N_OUT):
                """f32 weight slice for contraction chunk c."""
                if c < KH:
                    return wlo[:, c, lo:hi]
                r = c - KH
                return whi[:, r * N_OUT + lo : r * N_OUT + hi]

            def wqsl(c, lo=0, hi=N_OUT):
                if c < KH:
                    return wqlo[:, c, lo:hi]
                r = c - KH
                return wqhi[:, r * N_OUT + lo : r * N_OUT + hi]

            # ---- doorbells first on all three queues ----
            nc.sync.dma_start(out=ident, in_=i_h[:, :])
            HWH = KH * N_OUT // 2
            nc.gpsimd.dma_start(out=whi[:, 0:HWH], in_=whi_ap[1, :, 0:HWH])
            nc.gpsimd.dma_start(out=whi[:, HWH:], in_=whi_ap[1, :, HWH:])
            nc.sync.dma_start(out=wlo[:, 0, :], in_=wlo_ap[:, 0, :])
            nc.scalar.dma_start(out=wlo[:, 1, :], in_=wlo_ap[:, 1, :])
            nc.scalar.dma_start(out=wlo[:, 2, :], in_=wlo_ap[:, 2, :])
            nc.gpsimd.dma_start(out=wlo[:, 3, :], in_=wlo_ap[:, 3, :])
            nc.vector.memset(ones128, 1.0)
            nc.vector.memset(ones1, 1.0)

            cur_group = [None]

            def emit_x_load(t):
                if t < 4:
                    x32 = x1pool.tile([P, N_FEAT], f32, tag="x32")
                    nc.gpsimd.dma_start(out=x32, in_=xt_ap[t, :, :])
                    return x32[:, :]
                g, r = divmod(t, 4)
                if r == 0:
                    xg = xgpool.tile([P, 4 * N_FEAT], f32, tag="xg", name=f"xg{g}")
                    nc.gpsimd.dma_start(out=xg, in_=xg_ap[g, :, :])
                    cur_group[0] = xg
                return cur_group[0][:, (t % 4) * N_FEAT : (t % 4 + 1) * N_FEAT]

            def emit_cast(src):
                xb = bpool.tile([P, N_FEAT], bf16, tag="xb")
                nc.vector.tensor_copy(out=xb, in_=src)
                return xb

            def emit_T(xb):
                xTps = ps1pool.tile([P, KC, P], bf16, tag="xTps")
                for c in range(KH):
                    nc.tensor.transpose(
                        xTps[:, c, :], xb[:, c * P : (c + 1) * P], ident
                    )
                # chunks 4-7 contract features {512 + 4m + r}: stride-4 read
                xbhi = xb[:, 512:1024].rearrange("q (m r) -> q r m", r=4)
                for r in range(4):
                    nc.tensor.transpose(xTps[:, KH + r, :], xbhi[:, r, :], ident)
                return xTps

            def emit_evac(xTps):
                xT = tpool.tile([P, KC, P], bf16, tag="xT")
                nc.vector.tensor_copy(out=xT, in_=xTps)
                return xT

            def emit_warm(n):
                warm_ps = ps1pool.tile([P, P], bf16, tag="xTps")
                for _ in range(n):
                    nc.tensor.transpose(warm_ps, ident, ident)

            def emit_mm(ps, xT):
                for c in range(KC):
                    for h in range(2):
                        nc.tensor.matmul(
                            ps[:, h * 512 : (h + 1) * 512],
                            xT[:, c, :],
                            wqsl(c, h * 512, (h + 1) * 512),
                            start=(c == 0),
                            stop=(c == KC - 1),
                        )

            def emit_out(t, ps, tail=False):
                o = opool.tile([P, N_OUT], f32, tag="o")
                if t < 4:
                    dst = ot_ap[t, :, :]
                else:
                    g, r = divmod(t, 4)
                    dst = og_ap[g, r, :, :]
                for h in range(2):
                    nc.scalar.activation(
                        out=o[:, h * 512 : (h + 1) * 512],
                        in_=ps[:, h * 512 : (h + 1) * 512],
                        func=mybir.ActivationFunctionType.Copy,
                        bias=0.0, scale=beta,
                    )
                if tail:
                    # partition-split across the sync queue and the (idle)
                    # gpsimd SWDGE queue; never ring bells on the busy ACT
                    nc.sync.dma_start(out=dst[0:64, :], in_=o[0:64, :])
                    nc.gpsimd.dma_start(out=dst[64:128, :], in_=o[64:128, :])
                else:
                    nc.sync.dma_start(out=dst, in_=o)

            # ---- x singles behind the rowblock weight on the fast queue,
            # loaded in feature-halves: the c0-c3 transposes (and with them
            # the first matmul) unlock ~2us after the first half lands ----
            def emit_half_chain(t):
                xa = x1pool.tile([P, 512], f32, tag="x32", name=f"xa{t}")
                nc.gpsimd.dma_start(out=xa, in_=xt_ap[t, :, 0:512])
                xbt = bpool.tile([P, N_FEAT], bf16, tag="xb")
                nc.vector.tensor_copy(out=xbt[:, 0:512], in_=xa)
                xTps = ps1pool.tile([P, KC, P], bf16, tag="xTps")
                for c in range(KH):
                    nc.tensor.transpose(
                        xTps[:, c, :], xbt[:, c * P : (c + 1) * P], ident
                    )
                xb_ = x1pool.tile([P, 512], f32, tag="x32", name=f"xb{t}")
                nc.gpsimd.dma_start(out=xb_, in_=xt_ap[t, :, 512:1024])
                nc.vector.tensor_copy(out=xbt[:, 512:1024], in_=xb_)
                xbhi = xbt[:, 512:1024].rearrange("q (m r) -> q r m", r=4)
                for r in range(4):
                    nc.tensor.transpose(xTps[:, KH + r, :], xbhi[:, r, :], ident)
                return emit_evac(xTps)

            srcs = {}
            xT_list = {}
            xT_list[0] = emit_half_chain(0)
            emit_warm(4)
            xT_list[1] = emit_half_chain(1)

            # ---- row sums in arrival order (lo chunks trickle in on the
            # HWDGE queues; the hi half lands all at once ~15.7us) ----
            for c in (0, 2, 4, 6):
                nc.vector.tensor_reduce(
                    wsum[:, c : c + 1], w32sl(c),
                    axis=mybir.AxisListType.X, op=mybir.AluOpType.add,
                )
            for c in (1, 3, 5, 7):
                nc.scalar.activation(
                    out=wqsl(c), in_=w32sl(c),
                    func=mybir.ActivationFunctionType.Copy,
                    bias=0.0, scale=1.0,
                    accum_out=wsum[:, c : c + 1],
                )
            # tile 2's transposes fill the PE hole before the mean matmul
            xT_list[2] = emit_half_chain(2)
            nc.vector.tensor_reduce(
                ssum, wsum, axis=mybir.AxisListType.X, op=mybir.AluOpType.add
            )
            na_ps = ps1pool.tile([P, 1], f32, tag="xTps")
            nc.tensor.matmul(na_ps, ones128, ssum, start=True, stop=True)
            nc.vector.tensor_scalar_mul(
                neg_a, na_ps, -1.0 / float(N_FEAT * N_OUT)
            )

            # ---- signs; chunk 0 in halves for a faster first unlock ----
            nc.scalar.activation(
                out=wqsl(0, 0, 512), in_=w32sl(0, 0, 512),
                func=mybir.ActivationFunctionType.Sign, bias=neg_a, scale=1.0,
            )
            nc.scalar.activation(
                out=wqsl(0, 512, 1024), in_=w32sl(0, 512, 1024),
                func=mybir.ActivationFunctionType.Sign, bias=neg_a, scale=1.0,
            )
            for c in range(1, KC):
                nc.scalar.activation(
                    out=wqsl(c), in_=w32sl(c),
                    func=mybir.ActivationFunctionType.Sign, bias=neg_a, scale=1.0,
                )

            # ---- warm matmuls: chunk-major over tiles 0-2 so each sign
            # feeds ~1.28us of PE work; tile 3's transposes slot into the
            # sign-gated bubbles ----
            ps_w = [
                pspool.tile([P, N_OUT], f32, tag="ps", name=f"ps_w{i}")
                for i in range(NW)
            ]

            def warm_mm(c0, c1):
                for c in range(c0, c1):
                    for h in range(2):
                        for ti in range(NW):
                            nc.tensor.matmul(
                                ps_w[ti][:, h * 512 : (h + 1) * 512],
                                xT_list[ti][:, c, :],
                                wqsl(c, h * 512, (h + 1) * 512),
                                start=(c == 0),
                                stop=(c == KC - 1),
                            )

            warm_mm(0, 3)
            xT_list[3] = emit_half_chain(3)
            warm_mm(3, 6)



            # ---- last two chunks tile-major with outputs interleaved so
            # PSUM frees as early as possible ----
            for ti in range(NW):
                for c in (6, 7):
                    for h in range(2):
                        nc.tensor.matmul(
                            ps_w[ti][:, h * 512 : (h + 1) * 512],
                            xT_list[ti][:, c, :],
                            wqsl(c, h * 512, (h + 1) * 512),
                            start=False,
                            stop=(c == KC - 1),
                        )
                emit_out(ti, ps_w[ti])

            xT_list[4] = emit_evac(emit_T(emit_cast(emit_x_load(4))))
            xT_list[5] = emit_evac(emit_T(emit_cast(emit_x_load(5))))

            # ---- beta = max|w| (needed only by the first output copy) ----
            for c in range(KC):
                nc.vector.tensor_reduce(
                    bmax[:, c : c + 1], w32sl(c),
                    axis=mybir.AxisListType.X, op=mybir.AluOpType.max,
                    apply_absolute_value=True,
                )
            nc.vector.tensor_reduce(
                bmax1, bmax, axis=mybir.AxisListType.X, op=mybir.AluOpType.max
            )
            nc.gpsimd.tensor_reduce(
                pack2[:, 1:2], bmax1, axis=mybir.AxisListType.C,
                op=mybir.AluOpType.max,
            )
            b_ps = ps1pool.tile([P, 1], f32, tag="xTps")
            nc.tensor.matmul(b_ps, ones1, pack2[:, 1:2], start=True, stop=True)
            nc.vector.tensor_copy(out=beta, in_=b_ps)



            # ---- steady loop: PE stream is [T8(t+3), MM16(t)] ----
            for t in range(NW, T):
                if t + 3 < T:
                    if t + 3 not in srcs:
                        srcs[t + 3] = emit_x_load(t + 3)
                    xT_list[t + 3] = emit_evac(emit_T(emit_cast(srcs.pop(t + 3))))
                xT = xT_list.pop(t)
                ps = pspool.tile([P, N_OUT], f32, tag="ps")
                if t == T - 1:
                    # final tile: finish the h0 half completely first so its
                    # copy+store overlap the h1 matmuls; stores split 4 ways
                    o = opool.tile([P, N_OUT], f32, tag="o")
                    g, r = divmod(t, 4)
                    dst = og_ap[g, r, :, :]
                    for h in range(2):
                        for c in range(KC):
                            nc.tensor.matmul(
                                ps[:, h * 512 : (h + 1) * 512],
                                xT[:, c, :],
                                wqsl(c, h * 512, (h + 1) * 512),
                                start=(c == 0),
                                stop=(c == KC - 1),
                            )
                        nc.scalar.activation(
                            out=o[:, h * 512 : (h + 1) * 512],
                            in_=ps[:, h * 512 : (h + 1) * 512],
                            func=mybir.ActivationFunctionType.Copy,
                            bias=0.0, scale=beta,
                        )
                        nc.sync.dma_start(
                            out=dst[0:64, h * 512 : (h + 1) * 512],
                            in_=o[0:64, h * 512 : (h + 1) * 512],
                        )
                        nc.gpsimd.dma_start(
                            out=dst[64:128, h * 512 : (h + 1) * 512],
                            in_=o[64:128, h * 512 : (h + 1) * 512],
                        )
                else:
                    emit_mm(ps, xT)
                    emit_out(t, ps, tail=(t >= T - 3))

    return nc


def _get_nc(rows_per_core: int):
    if rows_per_core not in _NC_CACHE:
        _NC_CACHE[rows_per_core] = _build_nc(rows_per_core)
    return _NC_CACHE[rows_per_core]


def run(x, weight, trace=False, trace_cores=None):
    """Run on 8 cores; returns (out, BassKernelResults)."""
    from concourse.bass_utils import run_bass_kernel_spmd

    import ml_dtypes

    x = np.ascontiguousarray(np.asarray(x, dtype=np.float32))
    weight = np.ascontiguousarray(np.asarray(weight, dtype=np.float32))
    ident = np.eye(P, dtype=ml_dtypes.bfloat16)
    n = x.shape[0]
    assert n % N_CORES == 0
    rpc = n // N_CORES
    nc = _get_nc(rpc)
    in_maps = [
        {"x": x[i * rpc : (i + 1) * rpc], "weight": weight, "ident": ident}
        for i in range(N_CORES)
    ]
    kwargs = {}
    if trace:
        kwargs["trace"] = True
        if trace_cores is not None:
            kwargs["trace_cores"] = trace_cores
    res = run_bass_kernel_spmd(nc, in_maps, core_ids=list(range(N_CORES)), **kwargs)
    out = np.concatenate([r["out"] for r in res.results], axis=0)
    return out, res


def kernel(x, weight):
    out, _ = run(x, weight)
    return out
